# revision 63
# baseline (speedup 1.0000x reference)
"""Trainium2 Bass kernel for nn_EncoderVidCRN (CRN video QA encoder).

Strategy: pure data parallel over batch B=128 across 8 NeuronCores (16 batch
rows per core). Weights are replicated, cast to bf16 on host, and shipped
pre-transposed into PE-stationary [K, M] layouts with the SBUF partition index
innermost so every device DMA is a plain contiguous [128, ...] copy.

All activations are kept feature-major on device ([d_feature -> partitions,
batch-cols -> free]), so every matmul is psum[M_out_feat, N_cols] =
W_T[K, M].T @ actT[K, N] with no transposes anywhere.

CRN subset means: the reference's rng subset choices are input-independent
(np.random.RandomState(0) at trace time) and replicated here exactly. Means
are computed as unnormalized bf16 subset sums on the vector engine (using a
full-sum minus complement when the complement is smaller), with the 1/|sel|
normalization folded into the g-half of each weight bank on the host.

ELU is composed as relu(x) + min(exp(x), 1) - 1 on ScalarE+VectorE.
"""

import functools
import itertools
import sys

import numpy as np

sys.path.insert(0, "/opt/trn_rl_repo")

import ml_dtypes  # noqa: E402

import concourse.bass as bass  # noqa: E402,F401
import concourse.mybir as mybir  # noqa: E402
import concourse.tile as tile  # noqa: E402
from concourse import bacc  # noqa: E402
from concourse.bass_utils import run_bass_kernel_spmd  # noqa: E402

BF = ml_dtypes.bfloat16
B, C, F, V, D = 128, 8, 16, 2048, 512
NCORES = 8
BS = B // NCORES      # 16 batch rows per core
J = BS * C            # 128 clip-level columns per core
T = F - 4             # 12 retained time slots
JV = BS * T           # 192 video-level columns per core

F32 = mybir.dt.float32
BF16 = mybir.dt.bfloat16
F8 = mybir.dt.float8e4
E4 = ml_dtypes.float8_e4m3
AF = mybir.ActivationFunctionType
OP = mybir.AluOpType
DR = mybir.MatmulPerfMode.DoubleRow

USE_F8 = True    # fp8e4 for wa, wm, wih, w1, w2, w3 (+app when USE_DR)
USE_DR = True    # DoubleRow fp8 matmuls for stage A (app + wa both fp8)
WDT = F8 if USE_F8 else BF16
WNP = E4 if USE_F8 else BF
ADT = F8 if USE_DR else BF16
ANP = E4 if USE_DR else BF

# ---------------------------------------------------------------- subsets


def _subsets():
    """Replicate the reference's rng sequence exactly (trace-time constant)."""
    rng = np.random.RandomState(0)
    out = []
    for n in (F, F - 2, C, C - 2):
        sels = []
        for scale_id in range(1, n - 1):
            scale = n - scale_id
            rels = list(itertools.combinations(range(n), scale))
            idx = rng.choice(len(rels), min(1, len(rels)), replace=False)
            sels.append(list(rels[int(idx[0])]))
        out.append(sels)
    return out


SELS_M, SELS_Q, SELS_VM, SELS_VQ = _subsets()


def _f_order():
    """Order appearance f-slots so the first-processed crn_m scales' subset
    members are produced in stage A's earliest chunks."""
    rng_order = []
    direct = [si for si in range(len(SELS_M)) if 16 - len(SELS_M[si]) + 1 >= len(SELS_M[si])]
    comp = [si for si in range(len(SELS_M)) if si not in direct]
    for si in direct + comp:
        for f in SELS_M[si]:
            if f not in rng_order:
                rng_order.append(f)
    for f in range(F):
        if f not in rng_order:
            rng_order.append(f)
    return rng_order


F_ORDER = _f_order()
F_POS = {f: i for i, f in enumerate(F_ORDER)}

# bias table layout (f32 [128, 240])
BOFF_A, BOFF_M, BOFF_Q, BOFF_VM, BOFF_G = 0, 4, 8, 12, 16
BOFF_1 = 32            # 14*4
BOFF_2 = 88            # 12*4
BOFF_G2 = 136          # 12*4
BOFF_3 = 184           # 6*4
BOFF_4 = 208           # 4*4
BOFF_G4 = 224          # 4*4
NBIAS = 240

# ---------------------------------------------------------------- device IR


def _gsum(nc, pool, slicer, n_obj, sel, S, shape, tag, eng=None, f8_out=None):
    """Unnormalized bf16 subset sum over object slices.

    slicer(i) -> AP of object i; S = precomputed full sum (or None).
    Uses S - complement when the complement is cheaper.  eng picks the
    engine (vector by default).  f8_out: extra fp8 tile — the final op of
    the chain writes it directly (accumulation stays bf16), replacing a
    separate cast instruction.
    """
    eng = eng or nc.vector
    in_set = set(sel)
    comp = [i for i in range(n_obj) if i not in in_set]
    use_comp = S is not None and len(comp) + 1 < len(sel)
    if not use_comp and len(sel) == 1:
        assert f8_out is None
        return slicer(sel[0])
    out = pool.tile(list(shape), BF16, tag=tag, name=f"gsum_{tag}")
    ops = ([(S, slicer(comp[0]))] + [(out, slicer(i)) for i in comp[1:]])         if use_comp else         ([(slicer(sel[0]), slicer(sel[1]))] + [(out, slicer(i)) for i in sel[2:]])
    op = eng.tensor_sub if use_comp else eng.tensor_add
    for k, (a, b) in enumerate(ops):
        dst = f8_out if (f8_out is not None and k == len(ops) - 1) else out
        op(dst, a, b)
    return out if f8_out is None else f8_out


def _bank_mm(nc, ps_list, wt, g, cond, koff_g, koff_c):
    """psum[m] += Wg[:,m].T @ g + Wc[:,m].T @ cond for the 4 output chunks."""
    for m in range(4):
        ps = ps_list[m]
        for kc in range(4):
            nc.tensor.matmul(ps, wt[:, koff_g + kc, m * 128:(m + 1) * 128],
                             g[:, kc, :], start=(kc == 0), stop=False)
        for kc in range(4):
            nc.tensor.matmul(ps, wt[:, koff_c + kc, m * 128:(m + 1) * 128],
                             cond[:, kc, :], start=False, stop=(kc == 3))


def _bank_mm_dr(nc, ps_list, wt, g8, cond8, koff_g, koff_c):
    """DoubleRow fp8 bank matmul: k-chunks paired, both operands fp8e4."""
    for m in range(4):
        ps = ps_list[m]
        for k in range(2):
            nc.tensor.matmul(
                ps, wt[:, koff_g + 2 * k:koff_g + 2 * k + 2, m * 128:(m + 1) * 128],
                g8[:, 2 * k:2 * k + 2, :], start=(k == 0), stop=False,
                perf_mode=DR)
        for k in range(2):
            nc.tensor.matmul(
                ps, wt[:, koff_c + 2 * k:koff_c + 2 * k + 2, m * 128:(m + 1) * 128],
                cond8[:, 2 * k:2 * k + 2, :], start=False, stop=(k == 1),
                perf_mode=DR)


def _elu_group(nc, tpool, ps_list, baps, dsts, cols, gate_list=None,
               neg_gbaps=None, wide_dst=None, view=None, relu_path=True,
               gate_ps_wide=None, neg_gbap_wide=None):
    """Fused ELU (+ optional sigmoid gate) for four [128, cols] psum slices.

    relu_path: elu(x) = relu(x) + (min(exp(x), 1) - 1) — ACT Relu per m +
    one wide bf16 DVE add (cheapest DVE).  Otherwise the STT form
    elu(x) = max(x, min(exp(x), 1) - 1) keeps work on DVE.  Sigmoid stays
    exp-composed so every ACT op lives in the exp_and_others table set.
    """
    t_e = tpool.tile([128, 4, cols], F32, tag="t_exp", name="t_e", bufs=2)
    for m in range(4):
        nc.scalar.activation(t_e[:, m, :], ps_list[m], AF.Exp, bias=baps[m])
    t_m = tpool.tile([128, 4, cols], BF16 if relu_path else F32, tag="t_min",
                     name="t_m", bufs=2)
    nc.vector.tensor_scalar(t_m, t_e, 1.0, -1.0, OP.min, OP.add)
    if relu_path:
        t_r = tpool.tile([128, 4, cols], BF16, tag="t_rel", name="t_r", bufs=2)
        for m in range(4):
            nc.scalar.activation(t_r[:, m, :], ps_list[m], AF.Relu, bias=baps[m])
    if gate_list is None:
        if relu_path:
            wide = dsts if isinstance(dsts, bass.AP) else None
            assert wide is not None
            nc.vector.tensor_add(wide, t_r, t_m)
        else:
            for m in range(4):
                nc.vector.scalar_tensor_tensor(dsts[m], ps_list[m], baps[m],
                                               t_m[:, m, :], OP.add, OP.max)
        return
    t_z = tpool.tile([128, 4, cols], BF16 if relu_path else F32, tag="t_z",
                     name="t_z", bufs=2)
    if relu_path:
        nc.vector.tensor_add(t_z, t_r, t_m)
    else:
        for m in range(4):
            nc.vector.scalar_tensor_tensor(t_z[:, m, :], ps_list[m], baps[m],
                                           t_m[:, m, :], OP.add, OP.max)
    t_d = tpool.tile([128, 4, cols], F32, tag="t_d", name="t_d", bufs=2)
    for m in range(4):
        nc.scalar.activation(t_d[:, m, :], gate_list[m], AF.Exp,
                             bias=neg_gbaps[m], scale=-1.0)
    nc.vector.tensor_scalar_add(t_d, t_d, 1.0)
    nc.vector.reciprocal(t_d, t_d)
    if view is None:
        view = lambda ap: ap
    nc.vector.tensor_tensor(wide_dst, view(t_z), view(t_d), OP.mult)


def _tree_sum(nc, pool, slicer, n, shape, tag, name):
    """Two-accumulator bf16 sum of n slices (halves the serial DVE chain)."""
    out = pool.tile(list(shape), BF16, tag=tag, name=name)
    half = pool.tile(list(shape), BF16, tag=tag + "_h", name=name + "_h")
    nc.vector.tensor_add(out, slicer(0), slicer(1))
    nc.vector.tensor_add(half, slicer(2), slicer(3))
    for i in range(4, n):
        t = out if i % 2 == 0 else half
        nc.vector.tensor_add(t, t, slicer(i))
    nc.vector.tensor_add(out, out, half)
    return out


@functools.lru_cache(maxsize=2)
def _program(debug=False):
    nc = bacc.Bacc("TRN2", target_bir_lowering=False, debug=False,
                   num_devices=NCORES)

    app_d = nc.dram_tensor("app", [128, 4, 16, 512], ADT, kind="ExternalInput")
    mot_d = nc.dram_tensor("mot", [128, 16, J], BF16, kind="ExternalInput")
    q_d = nc.dram_tensor("q", [128, 4, BS], BF16, kind="ExternalInput")
    wa_d = nc.dram_tensor("wa", [128, 16, 512], WDT, kind="ExternalInput")
    wm_d = nc.dram_tensor("wm", [128, 16, 512], WDT, kind="ExternalInput")
    wq_d = nc.dram_tensor("wq", [128, 4, 512], BF16, kind="ExternalInput")
    wvm_d = nc.dram_tensor("wvm", [128, 4, 512], BF16, kind="ExternalInput")
    wih_d = nc.dram_tensor("wih", [128, 16, 16, 128], WDT, kind="ExternalInput")
    whh_d = nc.dram_tensor("whh", [128, 4, 2048], BF16, kind="ExternalInput")
    w1_d = nc.dram_tensor("w1", [128, 14, 8, 512], WDT, kind="ExternalInput")
    w2_d = nc.dram_tensor("w2", [128, 12, 16, 512], WDT, kind="ExternalInput")
    w3_d = nc.dram_tensor("w3", [128, 6, 8, 512], WDT, kind="ExternalInput")
    w4_d = nc.dram_tensor("w4", [128, 4, 16, 512], BF16, kind="ExternalInput")
    bias_d = nc.dram_tensor("bias", [128, NBIAS], F32, kind="ExternalInput")
    out_d = nc.dram_tensor("out", [128, 4 * 4 * JV], F32, kind="ExternalOutput")
    out_v = out_d.ap().rearrange("p (d s j) -> p d s j", d=4, s=4)
    dbg = {}
    if debug:
        for nm, shape, dt in [("dbg_objsT", [128, 4 * F * J], BF16),
                              ("dbg_objs2T", [128, 4 * 14 * J], BF16),
                              ("dbg_clipT", [128, 4 * C * BS * T], BF16),
                              ("dbg_objs4T", [128, 4 * 6 * JV], BF16),
                              ("dbg_gx", [128, 16 * J], F32),
                              ("dbg_h", [128, 4 * BS], BF16),
                              ("dbg_condm", [128, 4 * J], BF16),
                              ("dbg_qp", [128, 4 * BS], BF16)]:
            dbg[nm] = nc.dram_tensor(nm, shape, dt, kind="ExternalOutput")

    nc._phases = []

    def _mark(name):
        nc._phases.append((name, int(nc.get_next_instruction_name()[2:])))

    with tile.TileContext(nc) as tc:
        # Pools form a strict stack (release order = reverse of allocation).
        perm = tc.alloc_tile_pool(name="perm", bufs=1)
        gpool = tc.alloc_tile_pool(name="gpool", bufs=4)
        tpool = tc.alloc_tile_pool(name="tmp", bufs=4)
        stream = tc.alloc_tile_pool(name="stream", bufs=4)
        p5 = tc.alloc_tile_pool(name="p5", bufs=1)        # clipT
        p4 = tc.alloc_tile_pool(name="p4", bufs=1)        # objs2T
        p3 = tc.alloc_tile_pool(name="p3", bufs=1)        # objsT, condm
        p0 = tc.alloc_tile_pool(name="p0", bufs=1)        # early consts
        pp_early = tc.alloc_tile_pool(name="ps_early", bufs=1, space="PSUM")

        _mark("consts")
        # ---------------- constant loads
        bias = perm.tile([128, NBIAS], F32, name="bias")
        nc.sync.dma_start(bias, bias_d[:])

        def bap(off):
            return bias[:, off:off + 1]

        motT = p0.tile([128, 16, J], BF16, name="motT")
        nc.sync.dma_start(motT, mot_d[:])
        qT = p0.tile([128, 4, BS], BF16, name="qT")
        nc.sync.dma_start(qT, q_d[:])
        wqt = p0.tile([128, 4, 512], BF16, name="wqt")
        nc.gpsimd.dma_start(wqt, wq_d[:])

        _mark("qproj_condm")
        # ---------------- q_proj  [128, 4, BS]
        psq = pp_early.tile([128, 4, BS], F32, tag="psq", name="psq")
        for m in range(4):
            for kc in range(4):
                nc.tensor.matmul(psq[:, m, :], wqt[:, kc, m * 128:(m + 1) * 128],
                                 qT[:, kc, :], start=(kc == 0), stop=(kc == 3))
        qp = perm.tile([128, 4, BS], BF16, name="qp")
        for m in range(4):
            nc.vector.tensor_scalar_add(qp[:, m, :], psq[:, m, :], bap(BOFF_Q + m))

        # ---------------- mot_proj -> cond_m  [128, 4, J]
        wmt_a = stream.tile([128, 8, 512], WDT, tag="wf8", name="wmt_a", bufs=4)
        nc.gpsimd.dma_start(wmt_a, wm_d[:, 0:8, :])
        wmt_b = stream.tile([128, 8, 512], WDT, tag="wf8", name="wmt_b", bufs=4)
        nc.gpsimd.dma_start(wmt_b, wm_d[:, 8:16, :])
        pscm = pp_early.tile([128, 4, J], F32, tag="pscm", name="pscm")
        for m in range(4):
            for kc in range(16):
                wmt = wmt_a if kc < 8 else wmt_b
                nc.tensor.matmul(pscm[:, m, :], wmt[:, kc % 8, m * 128:(m + 1) * 128],
                                 motT[:, kc, :], start=(kc == 0), stop=(kc == 15))
        condm = p3.tile([128, 4, J], F8, name="condm")
        for m in range(4):
            nc.vector.tensor_scalar_add(condm[:, m, :], pscm[:, m, :],
                                        bap(BOFF_M + m))

        # cond_q: q_proj broadcast over clips -> [128, 4, BS, C]
        condq = perm.tile([128, 4, BS, C], F8, name="condq")
        nc.vector.tensor_copy(condq, qp[:, :, :, None].to_broadcast([128, 4, BS, C]))
        condq_v = condq.rearrange("p d b c -> p d (b c)")
        qvc = perm.tile([128, 4, BS, T], BF16, name="qvc")
        nc.vector.tensor_copy(qvc, qp[:, :, :, None].to_broadcast([128, 4, BS, T]))
        qvc_v = qvc.rearrange("p d b t -> p d (b t)")
        pp_early.release()

        _mark("stageA")
        # ---------------- stage A: app_proj -> objsT [128, 4, F, J]
        p2 = tc.alloc_tile_pool(name="p2", bufs=1)
        apps = tc.alloc_tile_pool(name="apps", bufs=2)
        pp_a = tc.alloc_tile_pool(name="ps_a", bufs=2, space="PSUM")
        wat = p2.tile([128, 16, 512], WDT, name="wat")
        nc.gpsimd.dma_start(wat, wa_d[:])
        objsT = p3.tile([128, 4, F, J], BF16, name="objsT")
        for cc in range(4):
            xc = apps.tile([128, 16, 512], ADT, tag="app", name="xc")
            nc.sync.dma_start(xc, app_d[:, cc, :, :])
            for m in range(4):
                ps_a = pp_a.tile([128, 512], F32, tag="psA", name="ps_a")
                if USE_DR:
                    for kc in range(8):
                        nc.tensor.matmul(
                            ps_a, wat[:, 2 * kc:2 * kc + 2, m * 128:(m + 1) * 128],
                            xc[:, 2 * kc:2 * kc + 2, :], start=(kc == 0),
                            stop=(kc == 7), perf_mode=DR)
                else:
                    for kc in range(16):
                        nc.tensor.matmul(ps_a, wat[:, kc, m * 128:(m + 1) * 128],
                                         xc[:, kc, :], start=(kc == 0), stop=(kc == 15))
                dst = objsT[:, m, cc * 4:(cc + 1) * 4, :].rearrange("p f j -> p (f j)")
                nc.scalar.activation(dst, ps_a, AF.Identity, bias=bap(BOFF_A + m))
        if debug:
            nc.sync.dma_start(dbg["dbg_objsT"][:], objsT.rearrange("p a b c -> p (a b c)"))
        pp_a.release()
        apps.release()
        p2.release()

        _mark("crn_m")
        # ---------------- crn_m: objsT -> objs2T [128, 4, 14, J]
        pp_crn = tc.alloc_tile_pool(name="ps_crn", bufs=2, space="PSUM")
        s_m = _tree_sum(nc, p3, lambda f: objsT[:, :, f, :], F,
                        (128, 4, J), "s_m", "s_m")
        objs2T = p4.tile([128, 4, 14, J], BF16, name="objs2T")
        for si, sel in enumerate(SELS_M):
            w1t = stream.tile([128, 8, 512], WDT, tag="wf8", name="w1t", bufs=4)
            nc.gpsimd.dma_start(w1t, w1_d[:, si, :, :])
            g8t = gpool.tile([128, 4, J], F8, tag="g8_clip", name="g8", bufs=3)
            g8 = _gsum(nc, gpool, lambda f: objsT[:, :, f, :], F, sel, s_m,
                       (128, 4, J), "g_clip", f8_out=g8t)
            ps = pp_crn.tile([128, 4, J], F32, tag="psM", name="ps_m1", bufs=3)
            _bank_mm_dr(nc, [ps[:, m, :] for m in range(4)], w1t, g8, condm, 0, 4)
            _elu_group(nc, tpool, [ps[:, m, :] for m in range(4)],
                       [bap(BOFF_1 + si * 4 + m) for m in range(4)],
                       objs2T[:, :, si, :], J, relu_path=True)

        _mark("gatesx")
        # ---------------- LSTM x-gates: gx = W_ih @ motT + (b_ih + b_hh)
        # accumulation groups must be sequential per PSUM bank (start=True
        # clears has_written for the whole bank) -> mi-outer loop.
        wihs = tc.alloc_tile_pool(name="wihs", bufs=3)
        p1 = tc.alloc_tile_pool(name="p1", bufs=1)
        ppx = tc.alloc_tile_pool(name="ps_x", bufs=2, space="PSUM")
        whht = p1.tile([128, 4, 2048], BF16, name="whht")
        nc.gpsimd.dma_start(whht, whh_d[:])
        wvmt = p1.tile([128, 4, 512], BF16, name="wvmt")
        nc.gpsimd.dma_start(wvmt, wvm_d[:])
        gx = p1.tile([128, 16, J], F32, name="gx")
        for mi in range(16):
            wih_t = wihs.tile([128, 16, 128], WDT, tag="wih", name="wih_t")
            nc.gpsimd.dma_start(wih_t, wih_d[:, mi, :, :])
            psx = ppx.tile([128, J], F32, tag="psx", name="psx")
            for kc in range(16):
                nc.tensor.matmul(psx, wih_t[:, kc, :], motT[:, kc, :],
                                 start=(kc == 0), stop=(kc == 15))
            nc.scalar.activation(gx[:, mi, :], psx, AF.Identity, bias=bap(BOFF_G + mi))
        ppx.release()
        pp_r = tc.alloc_tile_pool(name="ps_r", bufs=2, space="PSUM")
        # view with the time step as an explicit axis: cols j = b*8 + c
        gxr = gx.rearrange("p m (b c) -> p m c b", c=C)

        _mark("lstm")
        # ---------------- LSTM recurrence (8 steps, h/c are [128, 4, BS])
        # gate banks host-permuted to [i, f, o, g] so sigmoid covers one
        # contiguous 12-chunk block and tanh the last 4.
        h_prev = None
        c_prev = None
        for t in range(C):
            xg = gxr[:, :, t, :]
            if t == 0:
                gates = xg
            else:
                psr = pp_r.tile([128, 16, BS], F32, tag="psr", name="psr")
                for mi in range(16):
                    for kc in range(4):
                        nc.tensor.matmul(psr[:, mi, :],
                                         whht[:, kc, mi * 128:(mi + 1) * 128],
                                         h_prev[:, kc, :],
                                         start=(kc == 0), stop=(kc == 3))
                gates = tpool.tile([128, 16, BS], F32, tag="lstm_g", name="lstm_g")
                nc.vector.tensor_add(gates, psr, xg)
            d_ifo = tpool.tile([128, 12, BS], F32, tag="difo", name="d_ifo")
            nc.scalar.activation(d_ifo, gates[:, 0:12, :], AF.Exp, scale=-1.0)
            nc.vector.tensor_scalar_add(d_ifo, d_ifo, 1.0)
            nc.vector.reciprocal(d_ifo, d_ifo)
            tan_g = tpool.tile([128, 4, BS], F32, tag="tg", name="tan_g")
            nc.scalar.activation(tan_g, gates[:, 12:16, :], AF.Tanh)
            ig = tpool.tile([128, 4, BS], F32, tag="ig", name="ig", bufs=2)
            nc.vector.tensor_tensor(ig, tan_g, d_ifo[:, 0:4, :], OP.mult)
            if t == 0:
                c_t = ig
            else:
                c_t = tpool.tile([128, 4, BS], F32, tag="c_t", name="c_t", bufs=2)
                fc = tpool.tile([128, 4, BS], F32, tag="fc", name="fc")
                nc.vector.tensor_tensor(fc, c_prev, d_ifo[:, 4:8, :], OP.mult)
                nc.vector.tensor_add(c_t, fc, ig)
            tan_c = tpool.tile([128, 4, BS], F32, tag="tanc", name="tan_c")
            nc.scalar.activation(tan_c, c_t, AF.Tanh)
            h_t = tpool.tile([128, 4, BS], BF16, tag="h_t", name="h_t", bufs=2)
            nc.vector.tensor_tensor(h_t, tan_c, d_ifo[:, 8:12, :], OP.mult)
            h_prev, c_prev = h_t, c_t

        # vm_proj -> video cond [128, 4, BS, T]
        psv = pp_r.tile([128, 4, BS], F32, tag="psv", name="psv", bufs=1)
        for m in range(4):
            for kc in range(4):
                nc.tensor.matmul(psv[:, m, :], wvmt[:, kc, m * 128:(m + 1) * 128],
                                 h_prev[:, kc, :], start=(kc == 0), stop=(kc == 3))
        vmp = p1.tile([128, 4, BS], BF16, name="vmp")
        for m in range(4):
            nc.vector.tensor_scalar_add(vmp[:, m, :], psv[:, m, :],
                                        bap(BOFF_VM + m))
        vmc = perm.tile([128, 4, BS, T], BF16, name="vmc")
        nc.vector.tensor_copy(vmc, vmp[:, :, :, None].to_broadcast([128, 4, BS, T]))
        vmc_v = vmc.rearrange("p d b t -> p d (b t)")
        if debug:
            nc.sync.dma_start(dbg["dbg_gx"][:], gx.rearrange("p a b -> p (a b)"))
            nc.sync.dma_start(dbg["dbg_h"][:], h_prev.rearrange("p a b -> p (a b)"))
            nc.gpsimd.dma_start(dbg["dbg_condm"][:], condm.rearrange("p a b -> p (a b)"))
            nc.sync.dma_start(dbg["dbg_qp"][:], qp.rearrange("p a b -> p (a b)"))
        pp_r.release()
        p1.release()
        wihs.release()

        _mark("crn_q")
        # ---------------- crn_q: objs2T -> clipT [128, 4, C, BS, T]
        if debug:
            nc.sync.dma_start(dbg["dbg_objs2T"][:], objs2T.rearrange("p a b c -> p (a b c)"))
        s_2 = _tree_sum(nc, p4, lambda s: objs2T[:, :, s, :], F - 2,
                        (128, 4, J), "s_2", "s_2")
        clipT = p5.tile([128, 4, C, BS, T], BF16, name="clipT")
        for si, sel in enumerate(SELS_Q):
            w2t = stream.tile([128, 8, 512], WDT, tag="wf8", name="w2t", bufs=4)
            nc.gpsimd.dma_start(w2t, w2_d[:, si, 0:8, :])
            w2g = stream.tile([128, 8, 512], WDT, tag="wf8", name="w2g", bufs=4)
            nc.gpsimd.dma_start(w2g, w2_d[:, si, 8:16, :])
            g8t = gpool.tile([128, 4, J], F8, tag="g8_clip", name="g8q", bufs=3)
            g8 = _gsum(nc, gpool, lambda s: objs2T[:, :, s, :], F - 2, sel, s_2,
                       (128, 4, J), "g_clip", f8_out=g8t)
            ps_m = pp_crn.tile([128, 4, J], F32, tag="psM", name="ps_q1", bufs=3)
            ps_g = pp_crn.tile([128, 4, J], F32, tag="psG", name="ps_q2")
            _bank_mm_dr(nc, [ps_m[:, m, :] for m in range(4)], w2t, g8, condq_v, 0, 4)
            _bank_mm_dr(nc, [ps_g[:, m, :] for m in range(4)], w2g, g8, condq_v, 0, 4)
            wide = clipT[:, :, :, :, si].rearrange("p d c b -> p d b c")
            _elu_group(nc, tpool, [ps_m[:, m, :] for m in range(4)],
                       [bap(BOFF_2 + si * 4 + m) for m in range(4)], None, J,
                       gate_list=[ps_g[:, m, :] for m in range(4)],
                       neg_gbaps=[bap(BOFF_G2 + si * 4 + m) for m in range(4)],
                       wide_dst=wide, relu_path=True,
                       view=lambda ap: ap.rearrange("p d (b c) -> p d b c", c=C))
        if debug:
            nc.sync.dma_start(dbg["dbg_clipT"][:], clipT.rearrange("p a b c d -> p (a b c d)"))
        pp_crn.release()
        p0.release()
        p3.release()
        p4.release()

        _mark("crn_vm")
        # ---------------- crn_vm: clipT -> objs4T [128, 4, 6, JV]
        pp_v = tc.alloc_tile_pool(name="ps_v", bufs=1, space="PSUM")

        def clip_slice(c):
            return clipT[:, :, c, :, :].rearrange("p d b t -> p d (b t)")

        s_3 = _tree_sum(nc, p5, clip_slice, C, (128, 4, JV), "s_3", "s_3")
        objs4T = perm.tile([128, 4, 6, JV], BF16, name="objs4T")
        for si, sel in enumerate(SELS_VM):
            w3t = stream.tile([128, 8, 512], WDT, tag="wf8", name="w3t", bufs=4)
            nc.gpsimd.dma_start(w3t, w3_d[:, si, :, :])
            g = _gsum(nc, gpool, clip_slice, C, sel, s_3, (128, 4, JV), "g_vid")
            ps0 = pp_v.tile([128, 2, JV], F32, tag="psV0", name="ps_vm0", bufs=2)
            ps1 = pp_v.tile([128, 2, JV], F32, tag="psV1", name="ps_vm1", bufs=2)
            ps_list = [ps0[:, 0, :], ps0[:, 1, :], ps1[:, 0, :], ps1[:, 1, :]]
            _bank_mm(nc, ps_list, w3t, g, vmc_v, 0, 4)
            _elu_group(nc, tpool, ps_list,
                       [bap(BOFF_3 + si * 4 + m) for m in range(4)],
                       objs4T[:, :, si, :], JV, relu_path=True)

        _mark("crn_vq")
        # ---------------- crn_vq: objs4T -> out
        if debug:
            nc.sync.dma_start(dbg["dbg_objs4T"][:], objs4T.rearrange("p a b c -> p (a b c)"))

        def o4_slice(s):
            return objs4T[:, :, s, :]

        s_4 = _tree_sum(nc, perm, o4_slice, C - 2, (128, 4, JV), "s_4", "s_4")
        for si, sel in enumerate(SELS_VQ):
            w4t = stream.tile([128, 8, 512], BF16, tag="crnw8", name="w4t", bufs=3)
            nc.gpsimd.dma_start(w4t, w4_d[:, si, 0:8, :])
            w4g = stream.tile([128, 8, 512], BF16, tag="crnw8", name="w4g", bufs=3)
            nc.gpsimd.dma_start(w4g, w4_d[:, si, 8:16, :])
            g = _gsum(nc, gpool, o4_slice, C - 2, sel, s_4, (128, 4, JV), "g_vid")
            ps0 = pp_v.tile([128, 2, JV], F32, tag="psV0", name="ps_vq0", bufs=2)
            ps1 = pp_v.tile([128, 2, JV], F32, tag="psV1", name="ps_vq1", bufs=2)
            pg0 = pp_v.tile([128, 2, JV], F32, tag="psV2", name="ps_vq2", bufs=2)
            pg1 = pp_v.tile([128, 2, JV], F32, tag="psV3", name="ps_vq3", bufs=2)
            ps_list = [ps0[:, 0, :], ps0[:, 1, :], ps1[:, 0, :], ps1[:, 1, :]]
            pg_list = [pg0[:, 0, :], pg0[:, 1, :], pg1[:, 0, :], pg1[:, 1, :]]
            _bank_mm(nc, ps_list, w4t, g, qvc_v, 0, 4)
            _bank_mm(nc, pg_list, w4g, g, qvc_v, 0, 4)
            ot4 = tpool.tile([128, 4, JV], F32, tag="ot", name="ot4", bufs=2)
            _elu_group(nc, tpool, ps_list,
                       [bap(BOFF_4 + si * 4 + m) for m in range(4)], None, JV,
                       gate_list=pg_list,
                       neg_gbaps=[bap(BOFF_G4 + si * 4 + m) for m in range(4)],
                       wide_dst=ot4, relu_path=False)
            nc.sync.dma_start(out_v[:, :, si, :], ot4)

        for pool in (pp_v, p5, stream, tpool, gpool, perm):
            pool.release()

    nc.compile()
    return nc


# ---------------------------------------------------------------- host side


def _to_kxm(w_t, kchunks, dt=BF):
    """[K, M] f32 -> [128, kchunks, M] with partition index innermost."""
    K, M = w_t.shape
    assert K == kchunks * 128
    return np.ascontiguousarray(
        w_t.reshape(kchunks, 128, M).transpose(1, 0, 2)).astype(dt)


def _bank_tensor(Ws, sels, gWs=None, dt=BF):
    """Stack per-scale CRN banks -> [128, S, H*4, 512] bf16.

    Halves order: [Wg/|sel|, Wc] (+ [gWg/|sel|, gWc] when gated); each half is
    the [2D, D] -> [D_in, D_out] transposed stationary operand.
    """
    per = []
    for si, sel in enumerate(sels):
        s_id = si + 1
        halves = [Ws[s_id][:, :D].T / len(sel), Ws[s_id][:, D:].T]
        if gWs is not None:
            halves += [gWs[s_id][:, :D].T / len(sel), gWs[s_id][:, D:].T]
        h = np.stack([np.asarray(x, np.float32) for x in halves])  # [H, 512, 512]
        H = h.shape[0]
        per.append(h.reshape(H, 4, 128, 512).transpose(2, 0, 1, 3)
                   .reshape(128, H * 4, 512))
    return np.ascontiguousarray(np.stack(per, axis=1)).astype(dt)


def _vec_to_pm(v, chunks):
    """[chunks*128] f32 -> [128, chunks] per-partition bias layout."""
    return np.ascontiguousarray(
        np.asarray(v, np.float32).reshape(chunks, 128).T)


@functools.lru_cache(maxsize=1)
def _static_prep_cache():
    return {}


def _prep_weights(inputs):
    w = {}
    w["wa"] = _to_kxm(np.asarray(inputs["Wa"], np.float32).T, 16, WNP)
    w["wm"] = _to_kxm(np.asarray(inputs["Wm"], np.float32).T, 16, WNP)
    w["wq"] = _to_kxm(np.asarray(inputs["Wq"], np.float32).T, 4)
    w["wvm"] = _to_kxm(np.asarray(inputs["Wvm"], np.float32).T, 4)
    # LSTM gate banks permuted [i, f, o, g] so sigmoid is one contiguous block
    perm = np.concatenate([np.arange(0, 1024), np.arange(1536, 2048),
                           np.arange(1024, 1536)])
    wih = _to_kxm(np.asarray(inputs["W_ih"], np.float32)[perm].T, 16, WNP)
    w["wih"] = np.ascontiguousarray(
        wih.reshape(128, 16, 16, 128).transpose(0, 2, 1, 3))  # [128, mi, kc, 128]
    w["whh"] = _to_kxm(np.asarray(inputs["W_hh"], np.float32)[perm].T, 4)
    w["w1"] = _bank_tensor(np.asarray(inputs["W1"], np.float32), SELS_M, dt=WNP)
    w["w2"] = _bank_tensor(np.asarray(inputs["W2"], np.float32), SELS_Q,
                           np.asarray(inputs["gW2"], np.float32), dt=WNP)
    w["w3"] = _bank_tensor(np.asarray(inputs["W3"], np.float32), SELS_VM, dt=WNP)
    w["w4"] = _bank_tensor(np.asarray(inputs["W4"], np.float32), SELS_VQ,
                           np.asarray(inputs["gW4"], np.float32))

    bias = np.zeros((128, NBIAS), np.float32)
    bias[:, BOFF_A:BOFF_A + 4] = _vec_to_pm(inputs["ba"], 4)
    bias[:, BOFF_M:BOFF_M + 4] = _vec_to_pm(inputs["bm"], 4)
    bias[:, BOFF_Q:BOFF_Q + 4] = _vec_to_pm(inputs["bq"], 4)
    bias[:, BOFF_VM:BOFF_VM + 4] = _vec_to_pm(inputs["bvm"], 4)
    bias[:, BOFF_G:BOFF_G + 16] = _vec_to_pm(
        (np.asarray(inputs["b_ih"], np.float32)
         + np.asarray(inputs["b_hh"], np.float32))[perm], 16)
    for si in range(len(SELS_M)):
        bias[:, BOFF_1 + si * 4:BOFF_1 + si * 4 + 4] = _vec_to_pm(inputs["b1"][si + 1], 4)
    for si in range(len(SELS_Q)):
        bias[:, BOFF_2 + si * 4:BOFF_2 + si * 4 + 4] = _vec_to_pm(inputs["b2"][si + 1], 4)
        bias[:, BOFF_G2 + si * 4:BOFF_G2 + si * 4 + 4] = _vec_to_pm(-np.asarray(inputs["gb2"][si + 1], np.float32), 4)
    for si in range(len(SELS_VM)):
        bias[:, BOFF_3 + si * 4:BOFF_3 + si * 4 + 4] = _vec_to_pm(inputs["b3"][si + 1], 4)
    for si in range(len(SELS_VQ)):
        bias[:, BOFF_4 + si * 4:BOFF_4 + si * 4 + 4] = _vec_to_pm(inputs["b4"][si + 1], 4)
        bias[:, BOFF_G4 + si * 4:BOFF_G4 + si * 4 + 4] = _vec_to_pm(-np.asarray(inputs["gb4"][si + 1], np.float32), 4)
    w["bias"] = bias
    return w


def _prep_core_inputs(inputs, core):
    b0 = core * BS
    app = np.asarray(inputs["appearance_video_feat"][b0:b0 + BS], np.float32)
    mot = np.asarray(inputs["motion_video_feat"][b0:b0 + BS], np.float32)
    q = np.asarray(inputs["question_embedding"][b0:b0 + BS], np.float32)
    # app [BS, C, F, V] -> [p, cc, kc, (f4 j)] with 4 f-slots per chunk
    app_t = app.transpose(3, 2, 0, 1).reshape(V, F, J)
    app_t = app_t.reshape(16, 128, F, J).transpose(1, 0, 2, 3)   # [p, kc, f, j]
    app_t = app_t.reshape(128, 16, 4, 4 * J).transpose(0, 2, 1, 3)  # [p, cc, kc, 512]
    # mot [BS, C, V] -> [p, kc, j]
    mot_t = mot.transpose(2, 0, 1).reshape(V, J).reshape(16, 128, J).transpose(1, 0, 2)
    # q [BS, D] -> [p, kc, b]
    q_t = q.T.reshape(4, 128, BS).transpose(1, 0, 2)
    return {
        "app": np.ascontiguousarray(app_t).astype(ANP),
        "mot": np.ascontiguousarray(mot_t).astype(BF),
        "q": np.ascontiguousarray(q_t).astype(BF),
    }


def _assemble(results):
    out = np.empty((B, (C - 4) * T, D), np.float32)
    for core in range(NCORES):
        r = results[core]["out"].reshape(128, 4, 4, BS, T)
        # [p, dc, s, b, t] -> [b, s, t, dc, p]
        o = r.transpose(3, 2, 4, 1, 0).reshape(BS, (C - 4) * T, D)
        out[core * BS:(core + 1) * BS] = o
    return out


def build_in_maps(**inputs):
    w = _prep_weights(inputs)
    in_maps = []
    for core in range(NCORES):
        m = dict(w)
        m.update(_prep_core_inputs(inputs, core))
        in_maps.append(m)
    return in_maps


def kernel(**inputs):
    nc = _program(False)
    in_maps = build_in_maps(**inputs)
    res = run_bass_kernel_spmd(nc, in_maps, list(range(NCORES)))
    return _assemble(res.results)


if __name__ == "__main__":
    import reference

    inputs = {k: np.asarray(v) for k, v in reference.setup_inputs().items()}
    out = kernel(**inputs)
    exp = np.asarray(reference.reference(**inputs))
    err = np.abs(out - exp).max() / np.abs(exp).max()
    print("Relative error:", err)

